# revision 8
# baseline (speedup 1.0000x reference)
"""Trainium2 Bass kernel for the CNN+GRU autoregressive forecaster.

Self-contained: hardcodes the problem shapes (B=512, SEQ=96, PRED=48, C=7,
D=128, KS=5) and the 8-core data-parallel sharding (64 batch elements per
core).

Structure of the device program (per core, SPMD):
  - Everything lives in [D=128 partitions, (position, batch)] column-major
    layouts ("p-major": column index = position*B + b).
  - The autoregressive feedback never materializes preds: the new embedded
    column is (W_val@fc_w) @ h + (W_val@fc_b + b_val) + temb_col.
  - Conv stack outputs for window-interior positions are window independent
    (windows differ only via zero padding at their edges), so conv1/2/3 are
    computed once into "global" buffers over the whole 144-position timeline
    (incrementally extended as predictions arrive) and only 12 edge
    positions per window are recomputed into a small ring.
  - The 48 GRU chains (windows) run software-pipelined, OFF=14 ticks apart.
    At each tick the ~7 active chains advance one timestep together:
    batched matmuls (gate weights x [active chains * 64] columns) and
    batched gate math.
"""

import sys

sys.path.insert(0, "/opt/trn_rl_repo")

import numpy as np
import ml_dtypes

BF16 = ml_dtypes.bfloat16


class Cfg:
    def __init__(self, T=96, NW=48, OFF=14, RING=8, h_fp32=True,
                 gate_f32=True, mt_f32=True, n_cores=8):
        self.T = T          # window length (SEQ_LEN)
        self.NW = NW        # number of windows (PRED_LEN)
        self.OFF = OFF      # tick offset between chain starts
        self.RING = RING    # edge ring slots
        self.C = 7
        self.D = 128
        self.KS = 5
        self.B = 64         # batch per core
        self.PAD = self.KS // 2
        self.L = T + NW     # global timeline length
        self.EL = 3 * self.PAD  # left edge size (conv3 positions differing from glob)
        self.ER = 3 * self.PAD  # right edge size
        self.NE = self.EL + self.ER  # ring entries per window
        self.h_fp32 = h_fp32
        self.gate_f32 = gate_f32
        self.mt_f32 = mt_f32
        self.n_cores = n_cores
        self.TICKS = OFF * (NW - 1) + T
        self.MAXA = (T + OFF - 1) // OFF  # max active chains

    def emap(self, t):
        if t < self.EL:
            return t
        assert t >= self.T - self.ER
        return self.NE - (self.T - t)


REAL = Cfg(OFF=12, RING=9, h_fp32=False, gate_f32=False)


# ---------------------------------------------------------------------------
# host-side data prep
# ---------------------------------------------------------------------------

def _np32(x):
    return np.asarray(x, dtype=np.float32)


def host_shared(cfg, inp):
    """Weight-derived arrays shared by all cores."""
    D, C, KS = cfg.D, cfg.C, cfg.KS
    W_val = _np32(inp["W_val"])          # [D, C]
    b_val = _np32(inp["b_val"])          # [D]
    fc_w = _np32(inp["fc_w"])            # [C, D]
    fc_b = _np32(inp["fc_b"])            # [C]
    gi = _np32(inp["gru_bi"])            # [3D]
    gh = _np32(inp["gru_bh"])            # [3D]

    convW = np.zeros((3, KS, D, D), dtype=BF16)
    for li, nm in enumerate(["conv1_w", "conv2_w", "conv3_w"]):
        w = _np32(inp[nm])               # [O, I, KS]
        for k in range(KS):
            convW[li, k] = w[:, :, k].T.astype(BF16)   # lhsT [I, O]

    wi = _np32(inp["gru_Wi"])            # [3D, D]
    wh = _np32(inp["gru_Wh"])
    wiT = np.zeros((3, D, D), dtype=BF16)
    whT = np.zeros((3, D, D), dtype=BF16)
    for g in range(3):
        wiT[g] = wi[g * D:(g + 1) * D, :].T.astype(BF16)
        whT[g] = wh[g * D:(g + 1) * D, :].T.astype(BF16)

    wvf = W_val @ fc_w                   # [D, D]
    bvf = W_val @ fc_b + b_val           # [D]

    biases = np.zeros((D, 10), dtype=np.float32)
    biases[:, 0] = b_val
    biases[:, 1] = _np32(inp["conv1_b"])
    biases[:, 2] = _np32(inp["conv2_b"])
    biases[:, 3] = _np32(inp["conv3_b"])
    biases[:, 4] = gi[0:D] + gh[0:D]         # sigmoid r bias
    biases[:, 5] = gi[D:2 * D] + gh[D:2 * D]  # sigmoid z bias
    biases[:, 6] = gh[2 * D:3 * D]            # bh_n (inside r*(...))
    biases[:, 7] = gi[2 * D:3 * D]            # bi_n (tanh bias)
    biases[:, 8] = bvf
    biases[:, 9] = -biases[:, 5]              # for z' = sigmoid(-(s_z + b_z))

    fdt = np.float32 if cfg.h_fp32 else BF16
    return {
        "wval": W_val.T.astype(np.float32).copy(),        # lhsT [C, D]
        "convW": convW.reshape(3 * KS * D, D).copy(),     # [15*128, 128] -> device [D, 15*D] by tap-major? see below
        "wiT": wiT,
        "whT": whT,
        "wvfT": wvf.T.astype(BF16).copy(),                # lhsT [D, D]
        "fcT": fc_w.T.astype(fdt).copy(),                 # lhsT [D, C]
        "biases": biases,
        "fcb": fc_b.reshape(C, 1).astype(np.float32).copy(),
    }


def host_temb(cfg, inp):
    """[Bfull, L, D] fp32 temporal embedding from y_mark."""
    ym = np.asarray(inp["y_mark"])
    hour = _np32(inp["hour_emb"])
    wday = _np32(inp["weekday_emb"])
    day = _np32(inp["day_emb"])
    mon = _np32(inp["month_emb"])
    temb = (hour[ym[:, :, 0]] + wday[ym[:, :, 1]]
            + day[ym[:, :, 2]] + mon[ym[:, :, 3]])
    return temb.astype(np.float32)


def host_core_inputs(cfg, inp, shared, temb, core):
    """Per-core input map."""
    B, T, L, C, D = cfg.B, cfg.T, cfg.L, cfg.C, cfg.D
    bsl = slice(core * B, (core + 1) * B)
    xe = _np32(inp["x_enc"])[bsl]                    # [B, T, C]
    xeT = np.ascontiguousarray(xe.transpose(2, 1, 0)).reshape(C, T * B)
    tb = temb[bsl]                                   # [B, L, D]
    tembT = np.ascontiguousarray(tb.transpose(2, 1, 0)).reshape(D, L * B)
    m = {
        "xeT": xeT.astype(np.float32),
        "tembT": tembT.astype(BF16),
    }
    for k, v in shared.items():
        if k == "convW":
            m[k] = np.ascontiguousarray(v.reshape(3 * cfg.KS, D, D)
                                        .transpose(1, 0, 2)).reshape(D, 3 * cfg.KS * D)
        elif k in ("wiT", "whT"):
            m[k] = np.ascontiguousarray(v.transpose(1, 0, 2)).reshape(D, 3 * D)
        else:
            m[k] = v
    return m


# ---------------------------------------------------------------------------
# device program
# ---------------------------------------------------------------------------

def build_program(cfg):
    import concourse.bass as bass
    import concourse.bacc as bacc
    import concourse.mybir as mybir
    import concourse.tile as tile

    f32 = mybir.dt.float32
    bf16 = mybir.dt.bfloat16
    AF = mybir.ActivationFunctionType
    ALU = mybir.AluOpType

    T, NW, OFF, RING = cfg.T, cfg.NW, cfg.OFF, cfg.RING
    C, D, KS, B, PAD = cfg.C, cfg.D, cfg.KS, cfg.B, cfg.PAD
    L, EL, ER, NE = cfg.L, cfg.EL, cfg.ER, cfg.NE
    MAXA = cfg.MAXA
    h_dt = f32 if cfg.h_fp32 else bf16
    g_dt = f32 if cfg.gate_f32 else bf16
    m_dt = f32 if cfg.mt_f32 else bf16
    u_dt = f32 if (cfg.h_fp32 or cfg.gate_f32) else bf16  # h-update intermediates

    # bias column indices
    EVB, C1B, C2B, C3B, SR, SZ, BHN, BIN, BVF, NSZ = range(10)

    nc = bacc.Bacc("TRN2", debug=False, num_devices=cfg.n_cores)

    d_xeT = nc.dram_tensor("xeT", [C, T * B], f32, kind="ExternalInput")
    d_tembT = nc.dram_tensor("tembT", [D, L * B], bf16, kind="ExternalInput")
    d_wval = nc.dram_tensor("wval", [C, D], f32, kind="ExternalInput")
    d_convW = nc.dram_tensor("convW", [D, 3 * KS * D], bf16, kind="ExternalInput")
    d_wiT = nc.dram_tensor("wiT", [D, 3 * D], bf16, kind="ExternalInput")
    d_whT = nc.dram_tensor("whT", [D, 3 * D], bf16, kind="ExternalInput")
    d_wvfT = nc.dram_tensor("wvfT", [D, D], bf16, kind="ExternalInput")
    d_fcT = nc.dram_tensor("fcT", [D, C], h_dt, kind="ExternalInput")
    d_biases = nc.dram_tensor("biases", [D, 10], f32, kind="ExternalInput")
    d_fcb = nc.dram_tensor("fcb", [C, 1], f32, kind="ExternalInput")
    d_out = nc.dram_tensor("outT", [C, NW * B], f32, kind="ExternalOutput")

    def cap(tile_ap, off, dims):
        """Custom AP relative to a pool tile: dims = [(step, count), ...]."""
        part = list(tile_ap.ap)[0]
        return bass.AP(tile_ap.tensor, tile_ap.offset + off, [part] + dims)

    with tile.TileContext(nc) as tc:
        with (
            tc.tile_pool(name="persist", bufs=1) as pp,
            tc.tile_pool(name="work", bufs=2) as wp,
            tc.tile_pool(name="ps2", bufs=2, space="PSUM") as ps2,
            tc.tile_pool(name="ps1", bufs=1, space="PSUM") as ps1,
        ):
            # ---------------- persistent tiles ----------------
            eg = pp.tile([D, L * B], bf16, tag="eg")
            c1g = pp.tile([D, L * B], bf16, tag="c1g")
            c2g = pp.tile([D, L * B], bf16, tag="c2g")
            c3g = pp.tile([D, L * B], bf16, tag="c3g")
            ering = pp.tile([D, RING * NE * B], bf16, tag="ering")
            H = pp.tile([D, NW * B], h_dt, tag="H")
            Hb = pp.tile([D, NW * B], bf16, tag="Hb", name="Hb") if cfg.h_fp32 else H
            ttail = pp.tile([D, NW * B], bf16, tag="ttail")
            xe = pp.tile([C, T * B], f32, tag="xe")
            wval = pp.tile([C, D], f32, tag="wval")
            cw = pp.tile([D, 3 * KS * D], bf16, tag="cw")
            wiT = pp.tile([D, 3 * D], bf16, tag="wiT")
            whT = pp.tile([D, 3 * D], bf16, tag="whT")
            wvfT = pp.tile([D, D], bf16, tag="wvfT")
            fcT = pp.tile([D, C], h_dt, tag="fcT")
            bias = pp.tile([D, 10], f32, tag="bias")
            fcb = pp.tile([C, 1], f32, tag="fcb")
            outsb = pp.tile([C, NW * B], f32, tag="outsb")

            nc.sync.dma_start(xe[:], d_xeT[:])
            nc.sync.dma_start(wval[:], d_wval[:])
            nc.sync.dma_start(cw[:], d_convW[:])
            nc.sync.dma_start(wiT[:], d_wiT[:])
            nc.sync.dma_start(whT[:], d_whT[:])
            nc.sync.dma_start(wvfT[:], d_wvfT[:])
            nc.sync.dma_start(fcT[:], d_fcT[:])
            nc.sync.dma_start(bias[:], d_biases[:])
            nc.sync.dma_start(fcb[:], d_fcb[:])
            nc.sync.dma_start(ttail[:], d_tembT[:, T * B:])

            nc.gpsimd.memset(H[:], 0.0)
            if cfg.h_fp32:
                nc.gpsimd.memset(Hb[:], 0.0)

            def bias_ap(i):
                return bias[:, i:i + 1]

            def conv_lhsT(layer, k):
                i = layer * KS + k
                return cw[:, i * D:(i + 1) * D]

            PSW = max(MAXA * B, 512)
            _ps_cycle = ["r", "z", "ghn"]
            _ps_i = [0]

            def next_ps(width):
                tag = _ps_cycle[_ps_i[0] % len(_ps_cycle)]
                _ps_i[0] += 1
                return ps2.tile([D, PSW], f32, tag=tag, name="ps_" + tag)

            def conv_group(ps, wcols, layer, t0, cnt, vlo, vhi, src_of):
                """Accumulate conv taps for output positions [t0, t0+cnt) into
                ps[:, :cnt*B]. src_of(t, n) -> rhs AP for n consecutive input
                positions starting at t. Valid input positions: [vlo, vhi]."""
                plan = []
                for k in [PAD] + [k for k in range(KS) if k != PAD]:
                    d = k - PAD
                    lo = max(t0, vlo - d)
                    hi = min(t0 + cnt, vhi - d + 1)
                    if hi > lo:
                        plan.append((k, lo, hi))
                assert plan[0][1] == t0 and plan[0][2] == t0 + cnt
                for i, (k, lo, hi) in enumerate(plan):
                    nc.tensor.matmul(
                        ps[:, (lo - t0) * B:(hi - t0) * B],
                        conv_lhsT(layer, k),
                        src_of(lo + k - PAD, hi - lo),
                        start=(i == 0), stop=(i == len(plan) - 1))

            def eg_of(p, n):
                return eg[:, p * B:(p + n) * B]

            def c1g_of(p, n):
                return c1g[:, p * B:(p + n) * B]

            def c2g_of(p, n):
                return c2g[:, p * B:(p + n) * B]

            # ---------------- init: value embedding for positions [0, T) ----
            GP = 512 // B  # positions per psum group
            for p0 in range(0, T, GP):
                cnt = min(GP, T - p0)
                pe = next_ps(cnt * B)
                nc.tensor.matmul(pe[:, :cnt * B], wval[:],
                                 xe[:, p0 * B:(p0 + cnt) * B],
                                 start=True, stop=True)
                tb = wp.tile([D, GP * B], bf16, tag="tstream")
                nc.sync.dma_start(tb[:, :cnt * B],
                                  d_tembT[:, p0 * B:(p0 + cnt) * B])
                nc.vector.scalar_tensor_tensor(
                    eg[:, p0 * B:(p0 + cnt) * B], pe[:, :cnt * B],
                    bias_ap(EVB), tb[:, :cnt * B], ALU.add, ALU.add)

            # ---------------- init: global convs over [0, T) ---------------
            def glob_conv(layer, dst, src_of, plo, phi, vlo, vhi, bcol):
                for p0 in range(plo, phi + 1, GP):
                    cnt = min(GP, phi + 1 - p0)
                    ps = next_ps(cnt * B)
                    conv_group(ps, cnt * B, layer, p0, cnt, vlo, vhi, src_of)
                    nc.scalar.activation(dst[:, p0 * B:(p0 + cnt) * B],
                                         ps[:, :cnt * B], AF.Relu,
                                         bias=bias_ap(bcol))

            glob_conv(0, c1g, eg_of, PAD, T - 1 - PAD, 0, T - 1, C1B)
            glob_conv(1, c2g, c1g_of, 2 * PAD, T - 1 - 2 * PAD,
                      PAD, T - 1 - PAD, C2B)
            glob_conv(2, c3g, c2g_of, 3 * PAD, T - 1 - 3 * PAD,
                      2 * PAD, T - 1 - 2 * PAD, C3B)

            # ---------------- edge computation ----------------------------
            S1L = EL + 2 * PAD   # conv1 span needed for a left edge
            S2L = EL + PAD

            def edge_left(w):
                """Window w conv3 outputs t in [0, EL) -> ring slot."""
                s1 = wp.tile([D, S1L * B], bf16, tag="s1")
                for t0 in range(0, S1L, GP):
                    cnt = min(GP, S1L - t0)
                    ps = next_ps(cnt * B)
                    conv_group(ps, cnt * B, 0, t0, cnt, 0, T - 1,
                               lambda t, n: eg[:, (w + t) * B:(w + t + n) * B])
                    nc.scalar.activation(s1[:, t0 * B:(t0 + cnt) * B],
                                         ps[:, :cnt * B], AF.Relu,
                                         bias=bias_ap(C1B))
                s2 = wp.tile([D, S2L * B], bf16, tag="s2")
                for t0 in range(0, S2L, GP):
                    cnt = min(GP, S2L - t0)
                    ps = next_ps(cnt * B)
                    conv_group(ps, cnt * B, 1, t0, cnt, 0, S1L - 1,
                               lambda t, n: s1[:, t * B:(t + n) * B])
                    nc.scalar.activation(s2[:, t0 * B:(t0 + cnt) * B],
                                         ps[:, :cnt * B], AF.Relu,
                                         bias=bias_ap(C2B))
                ps = next_ps(EL * B)
                conv_group(ps, EL * B, 2, 0, EL, 0, S2L - 1,
                           lambda t, n: s2[:, t * B:(t + n) * B])
                base = ((w % RING) * NE + 0) * B
                nc.scalar.activation(
                    cap(ering, base, [(1, EL * B)]),
                    ps[:, :EL * B], AF.Relu, bias=bias_ap(C3B))

            def edge_right(w):
                """Window w conv3 outputs t in [T-ER, T) -> ring slot."""
                t1lo = T - ER - 2 * PAD
                s1 = wp.tile([D, S1L * B], bf16, tag="s1r")
                for i0 in range(0, S1L, GP):
                    cnt = min(GP, S1L - i0)
                    ps = next_ps(cnt * B)
                    conv_group(ps, cnt * B, 0, t1lo + i0, cnt, 0, T - 1,
                               lambda t, n: eg[:, (w + t) * B:(w + t + n) * B])
                    nc.scalar.activation(s1[:, i0 * B:(i0 + cnt) * B],
                                         ps[:, :cnt * B], AF.Relu,
                                         bias=bias_ap(C1B))
                t2lo = T - ER - PAD
                s2 = wp.tile([D, S2L * B], bf16, tag="s2r")
                for i0 in range(0, S2L, GP):
                    cnt = min(GP, S2L - i0)
                    ps = next_ps(cnt * B)
                    conv_group(ps, cnt * B, 1, t2lo + i0, cnt,
                               t1lo, T - 1,
                               lambda t, n: s1[:, (t - t1lo) * B:(t - t1lo + n) * B])
                    nc.scalar.activation(s2[:, i0 * B:(i0 + cnt) * B],
                                         ps[:, :cnt * B], AF.Relu,
                                         bias=bias_ap(C2B))
                ps = next_ps(ER * B)
                conv_group(ps, ER * B, 2, T - ER, ER, t2lo, T - 1,
                           lambda t, n: s2[:, (t - t2lo) * B:(t - t2lo + n) * B])
                base = ((w % RING) * NE + EL) * B
                nc.scalar.activation(
                    cap(ering, base, [(1, ER * B)]),
                    ps[:, :ER * B], AF.Relu, bias=bias_ap(C3B))

            for w in range(min(RING, NW)):
                edge_left(w)
            edge_right(0)

            # ---------------- pipelined GRU ticks --------------------------
            def gx_segments(act, tau):
                """[(col0, ncols, rhs_builder)] covering the active chains."""
                segs = []
                i = 0
                while i < len(act):
                    w, t = act[i]
                    if EL <= t <= T - 1 - ER:
                        j = i
                        while (j + 1 < len(act)
                               and EL <= act[j + 1][1] <= T - 1 - ER):
                            j += 1
                        n = j - i + 1
                        base = (tau - (OFF - 1) * w) * B
                        if n == 1:
                            segs.append((i, n, c3g[:, base:base + B]))
                        else:
                            segs.append((i, n, cap(
                                c3g, base, [((OFF - 1) * B, n), (1, B)])))
                        i = j + 1
                    else:
                        base = ((w % RING) * NE + cfg.emap(t)) * B
                        segs.append((i, 1, cap(ering, base, [(1, B)])))
                        i += 1
                return segs

            fixup_at = {}
            if NW > 1:
                for v in range(NW - 1):
                    fixup_at[OFF * v + T - 1] = v

            def emit_group(act, tau):
                """One tick step for a (sub)set of active chains; chains in
                `act` are w-descending with constant w-step `ws`."""
                nA = len(act)
                W = nA * B
                ws = act[0][0] - act[1][0] if nA > 1 else 1
                whi = act[0][0]
                slo = NW - 1 - whi

                def h_ap(t):
                    if nA == 1:
                        return t[:, slo * B:(slo + 1) * B]
                    return cap(t, slo * B, [(ws * B, nA), (1, B)])

                # gx rhs segments
                segs = []
                i = 0
                while i < nA:
                    w, t = act[i]
                    if EL <= t <= T - 1 - ER:
                        j = i
                        while (j + 1 < nA
                               and EL <= act[j + 1][1] <= T - 1 - ER):
                            j += 1
                        n = j - i + 1
                        base = (tau - (OFF - 1) * w) * B
                        if n == 1:
                            segs.append((i, n, c3g[:, base:base + B]))
                        else:
                            segs.append((i, n, cap(
                                c3g, base, [(ws * (OFF - 1) * B, n), (1, B)])))
                        i = j + 1
                    else:
                        base = ((w % RING) * NE + cfg.emap(t)) * B
                        segs.append((i, 1, cap(ering, base, [(1, B)])))
                        i += 1

                pr = ps2.tile([D, PSW], f32, tag="r", name="pr")
                pz = ps2.tile([D, PSW], f32, tag="z", name="pz")
                pn = ps2.tile([D, PSW], f32, tag="ghn", name="pn")
                px = ps1.tile([D, PSW], f32, tag="gxn", name="px")

                # gx matmuls (h-independent)
                for g, ps in ((0, pr), (1, pz), (2, px)):
                    for si, (i0, n, rhs) in enumerate(segs):
                        nc.tensor.matmul(
                            ps[:, i0 * B:(i0 + n) * B],
                            wiT[:, g * D:(g + 1) * D], rhs,
                            start=(si == 0),
                            stop=(g == 2 and si == len(segs) - 1))
                hb_sl = h_ap(Hb)
                # recurrent matmuls: r first (heads the serial chain), n next
                nc.tensor.matmul(pr[:, :W], whT[:, 0:D], hb_sl,
                                 start=False, stop=True)
                nc.tensor.matmul(pn[:, :W], whT[:, 2 * D:3 * D], hb_sl,
                                 start=True, stop=True)
                nc.tensor.matmul(pz[:, :W], whT[:, D:2 * D], hb_sl,
                                 start=False, stop=True)

                rz = wp.tile([D, 2 * MAXA * B], g_dt, tag="rz")
                r_sl = rz[:, 0:W]
                z_sl = rz[:, MAXA * B:MAXA * B + W]
                h_sl = h_ap(H)
                # critical chain: sigmoid(r) -> m -> tt -> tanh -> q -> h'
                nc.scalar.activation(r_sl, pr[:, :W], AF.Sigmoid,
                                     bias=bias_ap(SR))
                m = wp.tile([D, MAXA * B], m_dt, tag="m")
                nc.vector.scalar_tensor_tensor(m[:, :W], pn[:, :W],
                                               bias_ap(BHN), r_sl,
                                               ALU.add, ALU.mult)
                tt = wp.tile([D, MAXA * B], m_dt, tag="tt")
                nc.vector.tensor_add(tt[:, :W], m[:, :W], px[:, :W])
                # off-chain: z, z' = 1-z, zh = z*h
                nc.scalar.activation(z_sl, pz[:, :W], AF.Sigmoid,
                                     bias=bias_ap(SZ))
                zp = wp.tile([D, MAXA * B], g_dt, tag="zp")
                nc.gpsimd.tensor_scalar(
                    out=zp[:, :W], in0=z_sl, scalar1=-1.0, scalar2=1.0,
                    op0=ALU.mult, op1=ALU.add)
                zh = wp.tile([D, MAXA * B], u_dt, tag="zh")
                nc.gpsimd.tensor_mul(zh[:, :W], z_sl, h_sl)
                n_t = wp.tile([D, MAXA * B], g_dt, tag="n")
                nc.scalar.activation(n_t[:, :W], tt[:, :W], AF.Tanh,
                                     bias=bias_ap(BIN))
                q_t = wp.tile([D, MAXA * B], u_dt, tag="q")
                nc.vector.tensor_mul(q_t[:, :W], zp[:, :W], n_t[:, :W])
                nc.vector.tensor_add(h_sl, q_t[:, :W], zh[:, :W])
                if cfg.h_fp32:
                    nc.vector.tensor_copy(hb_sl, h_sl)

            for tau in range(cfg.TICKS):
                whi = min(tau // OFF, NW - 1)
                wlo = max((tau - (T - 1) + OFF - 1) // OFF, 0)
                act = [(w, tau - OFF * w) for w in range(whi, wlo - 1, -1)]
                for grp in (0, 1):
                    act_g = [p for p in act if p[0] % 2 == grp]
                    if act_g:
                        emit_group(act_g, tau)

                # fixup after chain v finishes
                v = fixup_at.get(tau)
                if v is not None:
                    sv = NW - 1 - v
                    pe = ps1.tile([D, 512], f32, tag="conv")
                    nc.tensor.matmul(pe[:, :B], wvfT[:],
                                     Hb[:, sv * B:(sv + 1) * B],
                                     start=True, stop=True)
                    nc.vector.scalar_tensor_tensor(
                        eg[:, (T + v) * B:(T + v + 1) * B], pe[:, :B],
                        bias_ap(BVF), ttail[:, v * B:(v + 1) * B],
                        ALU.add, ALU.add)
                    # global conv extensions (one new position per layer)
                    for layer, dst, src_of, bcol in (
                            (0, c1g, eg_of, C1B), (1, c2g, c1g_of, C2B),
                            (2, c3g, c2g_of, C3B)):
                        p1 = T + v - (layer + 1) * PAD
                        ps = ps1.tile([D, 512], f32, tag="conv")
                        conv_group(ps, B, layer, p1, 1, 0, L, src_of)
                        nc.scalar.activation(dst[:, p1 * B:(p1 + 1) * B],
                                             ps[:, :B], AF.Relu,
                                             bias=bias_ap(bcol))
                    edge_right(v + 1)
                    if v + RING < NW:
                        edge_left(v + RING)

            # ---------------- final fc over all stashed h ------------------
            for c0 in range(0, NW * B, 512):
                cnt = min(512, NW * B - c0)
                pf = ps1.tile([C, 512], f32, tag="conv")
                nc.tensor.matmul(pf[:, :cnt], fcT[:], H[:, c0:c0 + cnt],
                                 start=True, stop=True)
                nc.scalar.activation(outsb[:, c0:c0 + cnt], pf[:, :cnt],
                                     AF.Identity, bias=fcb[:])
            nc.sync.dma_start(d_out[:], outsb[:])

    nc.compile()
    return nc


# ---------------------------------------------------------------------------
# top-level entry
# ---------------------------------------------------------------------------

_CACHE = {}


def _get_program(cfg):
    key = (cfg.T, cfg.NW, cfg.OFF, cfg.RING, cfg.h_fp32, cfg.gate_f32,
           cfg.mt_f32, cfg.n_cores)
    if key not in _CACHE:
        _CACHE[key] = build_program(cfg)
    return _CACHE[key]


def unshard(cfg, outs):
    """outs: list of per-core outT [C, NW*B] -> full [Bfull, NW, C]."""
    full = np.zeros((cfg.B * cfg.n_cores, cfg.NW, cfg.C), np.float32)
    for core, o in enumerate(outs):
        ot = np.asarray(o).reshape(cfg.C, cfg.NW, cfg.B)
        # slot s corresponds to window v = NW-1-s
        full[core * cfg.B:(core + 1) * cfg.B] = ot[:, ::-1, :].transpose(2, 1, 0)
    return full


def kernel(**inputs):
    from concourse.bass_utils import run_bass_kernel_spmd

    cfg = REAL
    nc = _get_program(cfg)
    shared = host_shared(cfg, inputs)
    temb = host_temb(cfg, inputs)
    in_maps = [host_core_inputs(cfg, inputs, shared, temb, c)
               for c in range(cfg.n_cores)]
    res = run_bass_kernel_spmd(nc, in_maps, list(range(cfg.n_cores)))
    outs = [res.results[c]["outT"] for c in range(cfg.n_cores)]
    return unshard(cfg, outs)


# revision 11
# speedup vs baseline: 1.2045x; 1.2045x over previous
"""Trainium2 Bass kernel for the CNN+GRU autoregressive forecaster.

Self-contained: hardcodes the problem shapes (B=512, SEQ=96, PRED=48, C=7,
D=128, KS=5) and the 8-core data-parallel sharding (64 batch elements per
core).

Structure of the device program (per core, SPMD):
  - Everything lives in [D=128 partitions, (position, batch)] column-major
    layouts ("p-major": column index = position*B + b).
  - The autoregressive feedback never materializes preds: the new embedded
    column is (W_val@fc_w) @ h + (W_val@fc_b + b_val) + temb_col.
  - Conv stack outputs for window-interior positions are window independent
    (windows differ only via zero padding at their edges), so conv1/2/3 are
    computed once into "global" buffers over the whole 144-position timeline
    (incrementally extended as predictions arrive) and only 12 edge
    positions per window are recomputed into a small ring.
  - The 48 GRU chains (windows) run software-pipelined, OFF=14 ticks apart.
    At each tick the ~7 active chains advance one timestep together:
    batched matmuls (gate weights x [active chains * 64] columns) and
    batched gate math.
"""

import sys

sys.path.insert(0, "/opt/trn_rl_repo")

import numpy as np
import ml_dtypes

BF16 = ml_dtypes.bfloat16


class Cfg:
    def __init__(self, T=96, NW=48, OFF=14, RING=8, h_fp32=True,
                 gate_f32=True, mt_f32=True, n_cores=8):
        self.T = T          # window length (SEQ_LEN)
        self.NW = NW        # number of windows (PRED_LEN)
        self.OFF = OFF      # tick offset between chain starts
        self.RING = RING    # edge ring slots
        self.C = 7
        self.D = 128
        self.KS = 5
        self.B = 64         # batch per core
        self.PAD = self.KS // 2
        self.L = T + NW     # global timeline length
        self.EL = 3 * self.PAD  # left edge size (conv3 positions differing from glob)
        self.ER = 3 * self.PAD  # right edge size
        self.NE = self.EL + self.ER  # ring entries per window
        self.h_fp32 = h_fp32
        self.gate_f32 = gate_f32
        self.mt_f32 = mt_f32
        self.n_cores = n_cores
        self.TICKS = OFF * (NW - 1) + T
        self.MAXA = (T + OFF - 1) // OFF  # max active chains

    def emap(self, t):
        if t < self.EL:
            return t
        assert t >= self.T - self.ER
        return self.NE - (self.T - t)


REAL = Cfg(OFF=8, RING=13, h_fp32=False, gate_f32=False)


# ---------------------------------------------------------------------------
# host-side data prep
# ---------------------------------------------------------------------------

def _np32(x):
    return np.asarray(x, dtype=np.float32)


def host_shared(cfg, inp):
    """Weight-derived arrays shared by all cores."""
    D, C, KS = cfg.D, cfg.C, cfg.KS
    W_val = _np32(inp["W_val"])          # [D, C]
    b_val = _np32(inp["b_val"])          # [D]
    fc_w = _np32(inp["fc_w"])            # [C, D]
    fc_b = _np32(inp["fc_b"])            # [C]
    gi = _np32(inp["gru_bi"])            # [3D]
    gh = _np32(inp["gru_bh"])            # [3D]

    convW = np.zeros((3, KS, D, D), dtype=BF16)
    for li, nm in enumerate(["conv1_w", "conv2_w", "conv3_w"]):
        w = _np32(inp[nm])               # [O, I, KS]
        for k in range(KS):
            convW[li, k] = w[:, :, k].T.astype(BF16)   # lhsT [I, O]

    wi = _np32(inp["gru_Wi"])            # [3D, D]
    wh = _np32(inp["gru_Wh"])
    wiT = np.zeros((3, D, D), dtype=BF16)
    whT = np.zeros((3, D, D), dtype=BF16)
    for g in range(3):
        wiT[g] = wi[g * D:(g + 1) * D, :].T.astype(BF16)
        whT[g] = wh[g * D:(g + 1) * D, :].T.astype(BF16)

    wvf = W_val @ fc_w                   # [D, D]
    bvf = W_val @ fc_b + b_val           # [D]

    biases = np.zeros((D, 10), dtype=np.float32)
    biases[:, 0] = b_val
    biases[:, 1] = _np32(inp["conv1_b"])
    biases[:, 2] = _np32(inp["conv2_b"])
    biases[:, 3] = _np32(inp["conv3_b"])
    biases[:, 4] = gi[0:D] + gh[0:D]         # sigmoid r bias
    biases[:, 5] = gi[D:2 * D] + gh[D:2 * D]  # sigmoid z bias
    biases[:, 6] = gh[2 * D:3 * D]            # bh_n (inside r*(...))
    biases[:, 7] = gi[2 * D:3 * D]            # bi_n (tanh bias)
    biases[:, 8] = bvf
    biases[:, 9] = -biases[:, 5]              # for z' = sigmoid(-(s_z + b_z))

    fdt = np.float32 if cfg.h_fp32 else BF16
    return {
        "wval": W_val.T.astype(np.float32).copy(),        # lhsT [C, D]
        "convW": convW.reshape(3 * KS * D, D).copy(),     # [15*128, 128] -> device [D, 15*D] by tap-major? see below
        "wiT": wiT,
        "whT": whT,
        "wvfT": wvf.T.astype(BF16).copy(),                # lhsT [D, D]
        "fcT": fc_w.T.astype(fdt).copy(),                 # lhsT [D, C]
        "biases": biases,
        "fcb": fc_b.reshape(C, 1).astype(np.float32).copy(),
    }


def host_temb(cfg, inp):
    """[Bfull, L, D] fp32 temporal embedding from y_mark."""
    ym = np.asarray(inp["y_mark"])
    hour = _np32(inp["hour_emb"])
    wday = _np32(inp["weekday_emb"])
    day = _np32(inp["day_emb"])
    mon = _np32(inp["month_emb"])
    temb = (hour[ym[:, :, 0]] + wday[ym[:, :, 1]]
            + day[ym[:, :, 2]] + mon[ym[:, :, 3]])
    return temb.astype(np.float32)


def host_core_inputs(cfg, inp, shared, temb, core):
    """Per-core input map."""
    B, T, L, C, D = cfg.B, cfg.T, cfg.L, cfg.C, cfg.D
    bsl = slice(core * B, (core + 1) * B)
    xe = _np32(inp["x_enc"])[bsl]                    # [B, T, C]
    xeT = np.ascontiguousarray(xe.transpose(2, 1, 0)).reshape(C, T * B)
    tb = temb[bsl]                                   # [B, L, D]
    tembT = np.ascontiguousarray(tb.transpose(2, 1, 0)).reshape(D, L * B)
    m = {
        "xeT": xeT.astype(np.float32),
        "tembT": tembT.astype(BF16),
    }
    for k, v in shared.items():
        if k == "convW":
            m[k] = np.ascontiguousarray(v.reshape(3 * cfg.KS, D, D)
                                        .transpose(1, 0, 2)).reshape(D, 3 * cfg.KS * D)
        elif k in ("wiT", "whT"):
            m[k] = np.ascontiguousarray(v.transpose(1, 0, 2)).reshape(D, 3 * D)
        else:
            m[k] = v
    return m


# ---------------------------------------------------------------------------
# device program
# ---------------------------------------------------------------------------

def build_program(cfg):
    import concourse.bass as bass
    import concourse.bacc as bacc
    import concourse.mybir as mybir
    import concourse.tile as tile

    f32 = mybir.dt.float32
    bf16 = mybir.dt.bfloat16
    AF = mybir.ActivationFunctionType
    ALU = mybir.AluOpType

    T, NW, OFF, RING = cfg.T, cfg.NW, cfg.OFF, cfg.RING
    C, D, KS, B, PAD = cfg.C, cfg.D, cfg.KS, cfg.B, cfg.PAD
    L, EL, ER, NE = cfg.L, cfg.EL, cfg.ER, cfg.NE
    MAXA = cfg.MAXA
    h_dt = f32 if cfg.h_fp32 else bf16
    g_dt = f32 if cfg.gate_f32 else bf16
    m_dt = f32 if cfg.mt_f32 else bf16
    u_dt = f32 if (cfg.h_fp32 or cfg.gate_f32) else bf16  # h-update intermediates

    # bias column indices
    EVB, C1B, C2B, C3B, SR, SZ, BHN, BIN, BVF, NSZ = range(10)

    nc = bacc.Bacc("TRN2", debug=False, num_devices=cfg.n_cores)

    d_xeT = nc.dram_tensor("xeT", [C, T * B], f32, kind="ExternalInput")
    d_tembT = nc.dram_tensor("tembT", [D, L * B], bf16, kind="ExternalInput")
    d_wval = nc.dram_tensor("wval", [C, D], f32, kind="ExternalInput")
    d_convW = nc.dram_tensor("convW", [D, 3 * KS * D], bf16, kind="ExternalInput")
    d_wiT = nc.dram_tensor("wiT", [D, 3 * D], bf16, kind="ExternalInput")
    d_whT = nc.dram_tensor("whT", [D, 3 * D], bf16, kind="ExternalInput")
    d_wvfT = nc.dram_tensor("wvfT", [D, D], bf16, kind="ExternalInput")
    d_fcT = nc.dram_tensor("fcT", [D, C], h_dt, kind="ExternalInput")
    d_biases = nc.dram_tensor("biases", [D, 10], f32, kind="ExternalInput")
    d_fcb = nc.dram_tensor("fcb", [C, 1], f32, kind="ExternalInput")
    d_out = nc.dram_tensor("outT", [C, NW * B], f32, kind="ExternalOutput")

    def cap(tile_ap, off, dims):
        """Custom AP relative to a pool tile: dims = [(step, count), ...]."""
        part = list(tile_ap.ap)[0]
        return bass.AP(tile_ap.tensor, tile_ap.offset + off, [part] + dims)

    with tile.TileContext(nc) as tc:
        with (
            tc.tile_pool(name="persist", bufs=1) as pp,
            tc.tile_pool(name="work", bufs=2) as wp,
            tc.tile_pool(name="ps2", bufs=2, space="PSUM") as ps2,
            tc.tile_pool(name="ps1", bufs=1, space="PSUM") as ps1,
        ):
            # ---------------- persistent tiles ----------------
            eg = pp.tile([D, L * B], bf16, tag="eg")
            c1g = pp.tile([D, L * B], bf16, tag="c1g")
            c2g = pp.tile([D, L * B], bf16, tag="c2g")
            c3g = pp.tile([D, L * B], bf16, tag="c3g")
            ering = pp.tile([D, RING * NE * B], bf16, tag="ering")
            H = pp.tile([D, NW * B], h_dt, tag="H")
            Hb = pp.tile([D, NW * B], bf16, tag="Hb", name="Hb") if cfg.h_fp32 else H
            ttail = pp.tile([D, NW * B], bf16, tag="ttail")
            xe = pp.tile([C, T * B], f32, tag="xe")
            wval = pp.tile([C, D], f32, tag="wval")
            cw = pp.tile([D, 3 * KS * D], bf16, tag="cw")
            wiT = pp.tile([D, 3 * D], bf16, tag="wiT")
            whT = pp.tile([D, 3 * D], bf16, tag="whT")
            wvfT = pp.tile([D, D], bf16, tag="wvfT")
            fcT = pp.tile([D, C], h_dt, tag="fcT")
            bias = pp.tile([D, 10], f32, tag="bias")
            fcb = pp.tile([C, 1], f32, tag="fcb")
            outsb = pp.tile([C, NW * B], f32, tag="outsb")

            nc.sync.dma_start(xe[:], d_xeT[:])
            nc.sync.dma_start(wval[:], d_wval[:])
            nc.sync.dma_start(cw[:], d_convW[:])
            nc.sync.dma_start(wiT[:], d_wiT[:])
            nc.sync.dma_start(whT[:], d_whT[:])
            nc.sync.dma_start(wvfT[:], d_wvfT[:])
            nc.sync.dma_start(fcT[:], d_fcT[:])
            nc.sync.dma_start(bias[:], d_biases[:])
            nc.sync.dma_start(fcb[:], d_fcb[:])
            nc.sync.dma_start(ttail[:], d_tembT[:, T * B:])

            nc.gpsimd.memset(H[:], 0.0)
            if cfg.h_fp32:
                nc.gpsimd.memset(Hb[:], 0.0)

            def bias_ap(i):
                return bias[:, i:i + 1]

            def conv_lhsT(layer, k):
                i = layer * KS + k
                return cw[:, i * D:(i + 1) * D]

            PSW = max(((MAXA + 1) // 2) * B, 512)
            _ps_cycle = ["r", "z", "ghn"]
            _ps_i = [0]

            def next_ps(width):
                tag = _ps_cycle[_ps_i[0] % len(_ps_cycle)]
                _ps_i[0] += 1
                return ps2.tile([D, PSW], f32, tag=tag, name="ps_" + tag)

            def conv_group(ps, wcols, layer, t0, cnt, vlo, vhi, src_of):
                """Accumulate conv taps for output positions [t0, t0+cnt) into
                ps[:, :cnt*B]. src_of(t, n) -> rhs AP for n consecutive input
                positions starting at t. Valid input positions: [vlo, vhi]."""
                plan = []
                for k in [PAD] + [k for k in range(KS) if k != PAD]:
                    d = k - PAD
                    lo = max(t0, vlo - d)
                    hi = min(t0 + cnt, vhi - d + 1)
                    if hi > lo:
                        plan.append((k, lo, hi))
                assert plan[0][1] == t0 and plan[0][2] == t0 + cnt
                for i, (k, lo, hi) in enumerate(plan):
                    nc.tensor.matmul(
                        ps[:, (lo - t0) * B:(hi - t0) * B],
                        conv_lhsT(layer, k),
                        src_of(lo + k - PAD, hi - lo),
                        start=(i == 0), stop=(i == len(plan) - 1))

            def eg_of(p, n):
                return eg[:, p * B:(p + n) * B]

            def c1g_of(p, n):
                return c1g[:, p * B:(p + n) * B]

            def c2g_of(p, n):
                return c2g[:, p * B:(p + n) * B]

            # ---------------- init: value embedding for positions [0, T) ----
            GP = 512 // B  # positions per psum group
            for p0 in range(0, T, GP):
                cnt = min(GP, T - p0)
                pe = next_ps(cnt * B)
                nc.tensor.matmul(pe[:, :cnt * B], wval[:],
                                 xe[:, p0 * B:(p0 + cnt) * B],
                                 start=True, stop=True)
                tb = wp.tile([D, GP * B], bf16, tag="tstream")
                nc.sync.dma_start(tb[:, :cnt * B],
                                  d_tembT[:, p0 * B:(p0 + cnt) * B])
                nc.vector.scalar_tensor_tensor(
                    eg[:, p0 * B:(p0 + cnt) * B], pe[:, :cnt * B],
                    bias_ap(EVB), tb[:, :cnt * B], ALU.add, ALU.add)

            # ---------------- init: global convs over [0, T) ---------------
            def glob_conv(layer, dst, src_of, plo, phi, vlo, vhi, bcol):
                for p0 in range(plo, phi + 1, GP):
                    cnt = min(GP, phi + 1 - p0)
                    ps = next_ps(cnt * B)
                    conv_group(ps, cnt * B, layer, p0, cnt, vlo, vhi, src_of)
                    nc.scalar.activation(dst[:, p0 * B:(p0 + cnt) * B],
                                         ps[:, :cnt * B], AF.Relu,
                                         bias=bias_ap(bcol))

            glob_conv(0, c1g, eg_of, PAD, T - 1 - PAD, 0, T - 1, C1B)
            glob_conv(1, c2g, c1g_of, 2 * PAD, T - 1 - 2 * PAD,
                      PAD, T - 1 - PAD, C2B)
            glob_conv(2, c3g, c2g_of, 3 * PAD, T - 1 - 3 * PAD,
                      2 * PAD, T - 1 - 2 * PAD, C3B)

            # ---------------- edge computation ----------------------------
            S1L = EL + 2 * PAD   # conv1 span needed for a left edge
            S2L = EL + PAD

            def edge_left(w):
                """Window w conv3 outputs t in [0, EL) -> ring slot."""
                s1 = wp.tile([D, S1L * B], bf16, tag="s1")
                for t0 in range(0, S1L, GP):
                    cnt = min(GP, S1L - t0)
                    ps = next_ps(cnt * B)
                    conv_group(ps, cnt * B, 0, t0, cnt, 0, T - 1,
                               lambda t, n: eg[:, (w + t) * B:(w + t + n) * B])
                    nc.scalar.activation(s1[:, t0 * B:(t0 + cnt) * B],
                                         ps[:, :cnt * B], AF.Relu,
                                         bias=bias_ap(C1B))
                s2 = wp.tile([D, S2L * B], bf16, tag="s2")
                for t0 in range(0, S2L, GP):
                    cnt = min(GP, S2L - t0)
                    ps = next_ps(cnt * B)
                    conv_group(ps, cnt * B, 1, t0, cnt, 0, S1L - 1,
                               lambda t, n: s1[:, t * B:(t + n) * B])
                    nc.scalar.activation(s2[:, t0 * B:(t0 + cnt) * B],
                                         ps[:, :cnt * B], AF.Relu,
                                         bias=bias_ap(C2B))
                ps = next_ps(EL * B)
                conv_group(ps, EL * B, 2, 0, EL, 0, S2L - 1,
                           lambda t, n: s2[:, t * B:(t + n) * B])
                base = ((w % RING) * NE + 0) * B
                nc.scalar.activation(
                    cap(ering, base, [(1, EL * B)]),
                    ps[:, :EL * B], AF.Relu, bias=bias_ap(C3B))

            def edge_right(w):
                """Window w conv3 outputs t in [T-ER, T) -> ring slot."""
                t1lo = T - ER - 2 * PAD
                s1 = wp.tile([D, S1L * B], bf16, tag="s1r")
                for i0 in range(0, S1L, GP):
                    cnt = min(GP, S1L - i0)
                    ps = next_ps(cnt * B)
                    conv_group(ps, cnt * B, 0, t1lo + i0, cnt, 0, T - 1,
                               lambda t, n: eg[:, (w + t) * B:(w + t + n) * B])
                    nc.scalar.activation(s1[:, i0 * B:(i0 + cnt) * B],
                                         ps[:, :cnt * B], AF.Relu,
                                         bias=bias_ap(C1B))
                t2lo = T - ER - PAD
                s2 = wp.tile([D, S2L * B], bf16, tag="s2r")
                for i0 in range(0, S2L, GP):
                    cnt = min(GP, S2L - i0)
                    ps = next_ps(cnt * B)
                    conv_group(ps, cnt * B, 1, t2lo + i0, cnt,
                               t1lo, T - 1,
                               lambda t, n: s1[:, (t - t1lo) * B:(t - t1lo + n) * B])
                    nc.scalar.activation(s2[:, i0 * B:(i0 + cnt) * B],
                                         ps[:, :cnt * B], AF.Relu,
                                         bias=bias_ap(C2B))
                ps = next_ps(ER * B)
                conv_group(ps, ER * B, 2, T - ER, ER, t2lo, T - 1,
                           lambda t, n: s2[:, (t - t2lo) * B:(t - t2lo + n) * B])
                base = ((w % RING) * NE + EL) * B
                nc.scalar.activation(
                    cap(ering, base, [(1, ER * B)]),
                    ps[:, :ER * B], AF.Relu, bias=bias_ap(C3B))

            for w in range(min(RING, NW)):
                edge_left(w)
            edge_right(0)

            # ---------------- pipelined GRU ticks --------------------------
            def gx_segments(act, tau):
                """[(col0, ncols, rhs_builder)] covering the active chains."""
                segs = []
                i = 0
                while i < len(act):
                    w, t = act[i]
                    if EL <= t <= T - 1 - ER:
                        j = i
                        while (j + 1 < len(act)
                               and EL <= act[j + 1][1] <= T - 1 - ER):
                            j += 1
                        n = j - i + 1
                        base = (tau - (OFF - 1) * w) * B
                        if n == 1:
                            segs.append((i, n, c3g[:, base:base + B]))
                        else:
                            segs.append((i, n, cap(
                                c3g, base, [((OFF - 1) * B, n), (1, B)])))
                        i = j + 1
                    else:
                        base = ((w % RING) * NE + cfg.emap(t)) * B
                        segs.append((i, 1, cap(ering, base, [(1, B)])))
                        i += 1
                return segs

            fixup_at = {}
            if NW > 1:
                for v in range(NW - 1):
                    fixup_at[OFF * v + T - 1] = v

            def emit_group(act, tau):
                """One tick step for a (sub)set of active chains; chains in
                `act` are w-descending with constant w-step `ws`."""
                nA = len(act)
                W = nA * B
                ws = act[0][0] - act[1][0] if nA > 1 else 1
                whi = act[0][0]
                slo = NW - 1 - whi

                def h_ap(t):
                    if nA == 1:
                        return t[:, slo * B:(slo + 1) * B]
                    return cap(t, slo * B, [(ws * B, nA), (1, B)])

                # gx rhs segments
                segs = []
                i = 0
                while i < nA:
                    w, t = act[i]
                    if EL <= t <= T - 1 - ER:
                        j = i
                        while (j + 1 < nA
                               and EL <= act[j + 1][1] <= T - 1 - ER):
                            j += 1
                        n = j - i + 1
                        base = (tau - (OFF - 1) * w) * B
                        if n == 1:
                            segs.append((i, n, c3g[:, base:base + B]))
                        else:
                            segs.append((i, n, cap(
                                c3g, base, [(ws * (OFF - 1) * B, n), (1, B)])))
                        i = j + 1
                    else:
                        base = ((w % RING) * NE + cfg.emap(t)) * B
                        segs.append((i, 1, cap(ering, base, [(1, B)])))
                        i += 1

                pr = ps2.tile([D, PSW], f32, tag="r", name="pr")
                pz = ps2.tile([D, PSW], f32, tag="z", name="pz")
                pn = ps2.tile([D, PSW], f32, tag="ghn", name="pn")
                px = ps1.tile([D, PSW], f32, tag="gxn", name="px")

                # gx matmuls (h-independent)
                for g, ps in ((0, pr), (1, pz), (2, px)):
                    for si, (i0, n, rhs) in enumerate(segs):
                        nc.tensor.matmul(
                            ps[:, i0 * B:(i0 + n) * B],
                            wiT[:, g * D:(g + 1) * D], rhs,
                            start=(si == 0),
                            stop=(g == 2 and si == len(segs) - 1))
                hb_sl = h_ap(Hb)
                # recurrent matmuls: r first (heads the serial chain), n next
                nc.tensor.matmul(pr[:, :W], whT[:, 0:D], hb_sl,
                                 start=False, stop=True)
                nc.tensor.matmul(pn[:, :W], whT[:, 2 * D:3 * D], hb_sl,
                                 start=True, stop=True)
                nc.tensor.matmul(pz[:, :W], whT[:, D:2 * D], hb_sl,
                                 start=False, stop=True)

                rz = wp.tile([D, 2 * MAXA * B], g_dt, tag="rz")
                r_sl = rz[:, 0:W]
                z_sl = rz[:, MAXA * B:MAXA * B + W]
                h_sl = h_ap(H)
                # critical chain: sigmoid(r) -> m -> tt -> tanh -> q -> h'
                nc.scalar.activation(r_sl, pr[:, :W], AF.Sigmoid,
                                     bias=bias_ap(SR))
                m = wp.tile([D, MAXA * B], m_dt, tag="m")
                nc.vector.scalar_tensor_tensor(m[:, :W], pn[:, :W],
                                               bias_ap(BHN), r_sl,
                                               ALU.add, ALU.mult)
                tt = wp.tile([D, MAXA * B], m_dt, tag="tt")
                nc.vector.tensor_add(tt[:, :W], m[:, :W], px[:, :W])
                # off-chain: z, z' = 1-z, zh = z*h
                nc.scalar.activation(z_sl, pz[:, :W], AF.Sigmoid,
                                     bias=bias_ap(SZ))
                zp = wp.tile([D, MAXA * B], g_dt, tag="zp")
                nc.vector.tensor_scalar(
                    out=zp[:, :W], in0=z_sl, scalar1=-1.0, scalar2=1.0,
                    op0=ALU.mult, op1=ALU.add)
                zh = wp.tile([D, MAXA * B], u_dt, tag="zh")
                nc.vector.tensor_mul(zh[:, :W], z_sl, h_sl)
                n_t = wp.tile([D, MAXA * B], g_dt, tag="n")
                nc.scalar.activation(n_t[:, :W], tt[:, :W], AF.Tanh,
                                     bias=bias_ap(BIN))
                q_t = wp.tile([D, MAXA * B], u_dt, tag="q")
                nc.vector.tensor_mul(q_t[:, :W], zp[:, :W], n_t[:, :W])
                nc.vector.tensor_add(h_sl, q_t[:, :W], zh[:, :W])
                if cfg.h_fp32:
                    nc.vector.tensor_copy(hb_sl, h_sl)

            for tau in range(cfg.TICKS):
                whi = min(tau // OFF, NW - 1)
                wlo = max((tau - (T - 1) + OFF - 1) // OFF, 0)
                act = [(w, tau - OFF * w) for w in range(whi, wlo - 1, -1)]
                for grp in (0, 1):
                    act_g = [p for p in act if p[0] % 2 == grp]
                    if act_g:
                        emit_group(act_g, tau)

                # fixup after chain v finishes
                v = fixup_at.get(tau)
                if v is not None:
                    sv = NW - 1 - v
                    pe = ps1.tile([D, 512], f32, tag="conv")
                    nc.tensor.matmul(pe[:, :B], wvfT[:],
                                     Hb[:, sv * B:(sv + 1) * B],
                                     start=True, stop=True)
                    nc.vector.scalar_tensor_tensor(
                        eg[:, (T + v) * B:(T + v + 1) * B], pe[:, :B],
                        bias_ap(BVF), ttail[:, v * B:(v + 1) * B],
                        ALU.add, ALU.add)
                    # global conv extensions (one new position per layer)
                    for layer, dst, src_of, bcol in (
                            (0, c1g, eg_of, C1B), (1, c2g, c1g_of, C2B),
                            (2, c3g, c2g_of, C3B)):
                        p1 = T + v - (layer + 1) * PAD
                        ps = ps1.tile([D, 512], f32, tag="conv")
                        conv_group(ps, B, layer, p1, 1, 0, L, src_of)
                        nc.scalar.activation(dst[:, p1 * B:(p1 + 1) * B],
                                             ps[:, :B], AF.Relu,
                                             bias=bias_ap(bcol))
                    edge_right(v + 1)
                    if v + RING < NW:
                        edge_left(v + RING)

            # ---------------- final fc over all stashed h ------------------
            for c0 in range(0, NW * B, 512):
                cnt = min(512, NW * B - c0)
                pf = ps1.tile([C, 512], f32, tag="conv")
                nc.tensor.matmul(pf[:, :cnt], fcT[:], H[:, c0:c0 + cnt],
                                 start=True, stop=True)
                nc.scalar.activation(outsb[:, c0:c0 + cnt], pf[:, :cnt],
                                     AF.Identity, bias=fcb[:])
            nc.sync.dma_start(d_out[:], outsb[:])

    nc.compile()
    return nc


# ---------------------------------------------------------------------------
# top-level entry
# ---------------------------------------------------------------------------

_CACHE = {}


def _get_program(cfg):
    key = (cfg.T, cfg.NW, cfg.OFF, cfg.RING, cfg.h_fp32, cfg.gate_f32,
           cfg.mt_f32, cfg.n_cores)
    if key not in _CACHE:
        _CACHE[key] = build_program(cfg)
    return _CACHE[key]


def unshard(cfg, outs):
    """outs: list of per-core outT [C, NW*B] -> full [Bfull, NW, C]."""
    full = np.zeros((cfg.B * cfg.n_cores, cfg.NW, cfg.C), np.float32)
    for core, o in enumerate(outs):
        ot = np.asarray(o).reshape(cfg.C, cfg.NW, cfg.B)
        # slot s corresponds to window v = NW-1-s
        full[core * cfg.B:(core + 1) * cfg.B] = ot[:, ::-1, :].transpose(2, 1, 0)
    return full


def kernel(**inputs):
    from concourse.bass_utils import run_bass_kernel_spmd

    cfg = REAL
    nc = _get_program(cfg)
    shared = host_shared(cfg, inputs)
    temb = host_temb(cfg, inputs)
    in_maps = [host_core_inputs(cfg, inputs, shared, temb, c)
               for c in range(cfg.n_cores)]
    res = run_bass_kernel_spmd(nc, in_maps, list(range(cfg.n_cores)))
    outs = [res.results[c]["outT"] for c in range(cfg.n_cores)]
    return unshard(cfg, outs)


# revision 12
# speedup vs baseline: 1.3542x; 1.1243x over previous
"""Trainium2 Bass kernel for the CNN+GRU autoregressive forecaster.

Self-contained: hardcodes the problem shapes (B=512, SEQ=96, PRED=48, C=7,
D=128, KS=5) and the 8-core data-parallel sharding (64 batch elements per
core).

Structure of the device program (per core, SPMD):
  - Everything lives in [D=128 partitions, (position, batch)] column-major
    layouts ("p-major": column index = position*B + b).
  - The autoregressive feedback never materializes preds: the new embedded
    column is (W_val@fc_w) @ h + (W_val@fc_b + b_val) + temb_col.
  - Conv stack outputs for window-interior positions are window independent
    (windows differ only via zero padding at their edges), so conv1/2/3 are
    computed once into "global" buffers over the whole 144-position timeline
    (incrementally extended as predictions arrive) and only 12 edge
    positions per window are recomputed into a small ring.
  - The 48 GRU chains (windows) run software-pipelined, OFF=14 ticks apart.
    At each tick the ~7 active chains advance one timestep together:
    batched matmuls (gate weights x [active chains * 64] columns) and
    batched gate math.
"""

import sys

sys.path.insert(0, "/opt/trn_rl_repo")

import numpy as np
import ml_dtypes

BF16 = ml_dtypes.bfloat16


class Cfg:
    def __init__(self, T=96, NW=48, OFF=14, RING=8, h_fp32=True,
                 gate_f32=True, mt_f32=True, n_cores=8):
        self.T = T          # window length (SEQ_LEN)
        self.NW = NW        # number of windows (PRED_LEN)
        self.OFF = OFF      # tick offset between chain starts
        self.RING = RING    # edge ring slots
        self.C = 7
        self.D = 128
        self.KS = 5
        self.B = 64         # batch per core
        self.PAD = self.KS // 2
        self.L = T + NW     # global timeline length
        self.EL = 3 * self.PAD  # left edge size (conv3 positions differing from glob)
        self.ER = 3 * self.PAD  # right edge size
        self.NE = self.EL + self.ER  # ring entries per window
        self.h_fp32 = h_fp32
        self.gate_f32 = gate_f32
        self.mt_f32 = mt_f32
        self.n_cores = n_cores
        self.TICKS = OFF * (NW - 1) + T
        self.MAXA = (T + OFF - 1) // OFF  # max active chains

    def emap(self, t):
        if t < self.EL:
            return t
        assert t >= self.T - self.ER
        return self.NE - (self.T - t)


REAL = Cfg(OFF=8, RING=13, h_fp32=False, gate_f32=False)


# ---------------------------------------------------------------------------
# host-side data prep
# ---------------------------------------------------------------------------

def _np32(x):
    return np.asarray(x, dtype=np.float32)


def host_shared(cfg, inp):
    """Weight-derived arrays shared by all cores."""
    D, C, KS = cfg.D, cfg.C, cfg.KS
    W_val = _np32(inp["W_val"])          # [D, C]
    b_val = _np32(inp["b_val"])          # [D]
    fc_w = _np32(inp["fc_w"])            # [C, D]
    fc_b = _np32(inp["fc_b"])            # [C]
    gi = _np32(inp["gru_bi"])            # [3D]
    gh = _np32(inp["gru_bh"])            # [3D]

    convW = np.zeros((3, KS, D, D), dtype=BF16)
    for li, nm in enumerate(["conv1_w", "conv2_w", "conv3_w"]):
        w = _np32(inp[nm])               # [O, I, KS]
        for k in range(KS):
            convW[li, k] = w[:, :, k].T.astype(BF16)   # lhsT [I, O]

    wi = _np32(inp["gru_Wi"])            # [3D, D]
    wh = _np32(inp["gru_Wh"])
    wiT = np.zeros((3, D, D), dtype=BF16)
    whT = np.zeros((3, D, D), dtype=BF16)
    for g in range(3):
        wiT[g] = wi[g * D:(g + 1) * D, :].T.astype(BF16)
        whT[g] = wh[g * D:(g + 1) * D, :].T.astype(BF16)

    wvf = W_val @ fc_w                   # [D, D]
    bvf = W_val @ fc_b + b_val           # [D]

    biases = np.zeros((D, 10), dtype=np.float32)
    biases[:, 0] = b_val
    biases[:, 1] = _np32(inp["conv1_b"])
    biases[:, 2] = _np32(inp["conv2_b"])
    biases[:, 3] = _np32(inp["conv3_b"])
    biases[:, 4] = gi[0:D] + gh[0:D]         # sigmoid r bias
    biases[:, 5] = gi[D:2 * D] + gh[D:2 * D]  # sigmoid z bias
    biases[:, 6] = gh[2 * D:3 * D]            # bh_n (inside r*(...))
    biases[:, 7] = gi[2 * D:3 * D]            # bi_n (tanh bias)
    biases[:, 8] = bvf
    biases[:, 9] = -biases[:, 5]              # for z' = sigmoid(-(s_z + b_z))

    fdt = np.float32 if cfg.h_fp32 else BF16
    return {
        "wval": W_val.T.astype(np.float32).copy(),        # lhsT [C, D]
        "convW": convW.reshape(3 * KS * D, D).copy(),     # [15*128, 128] -> device [D, 15*D] by tap-major? see below
        "wiT": wiT,
        "whT": whT,
        "wvfT": wvf.T.astype(BF16).copy(),                # lhsT [D, D]
        "fcT": fc_w.T.astype(fdt).copy(),                 # lhsT [D, C]
        "biases": biases,
        "fcb": fc_b.reshape(C, 1).astype(np.float32).copy(),
    }


def host_temb(cfg, inp):
    """[Bfull, L, D] fp32 temporal embedding from y_mark."""
    ym = np.asarray(inp["y_mark"])
    hour = _np32(inp["hour_emb"])
    wday = _np32(inp["weekday_emb"])
    day = _np32(inp["day_emb"])
    mon = _np32(inp["month_emb"])
    temb = (hour[ym[:, :, 0]] + wday[ym[:, :, 1]]
            + day[ym[:, :, 2]] + mon[ym[:, :, 3]])
    return temb.astype(np.float32)


def host_core_inputs(cfg, inp, shared, temb, core):
    """Per-core input map."""
    B, T, L, C, D = cfg.B, cfg.T, cfg.L, cfg.C, cfg.D
    bsl = slice(core * B, (core + 1) * B)
    xe = _np32(inp["x_enc"])[bsl]                    # [B, T, C]
    xeT = np.ascontiguousarray(xe.transpose(2, 1, 0)).reshape(C, T * B)
    tb = temb[bsl]                                   # [B, L, D]
    tembT = np.ascontiguousarray(tb.transpose(2, 1, 0)).reshape(D, L * B)
    m = {
        "xeT": xeT.astype(np.float32),
        "tembT": tembT.astype(BF16),
    }
    for k, v in shared.items():
        if k == "convW":
            m[k] = np.ascontiguousarray(v.reshape(3 * cfg.KS, D, D)
                                        .transpose(1, 0, 2)).reshape(D, 3 * cfg.KS * D)
        elif k in ("wiT", "whT"):
            m[k] = np.ascontiguousarray(v.transpose(1, 0, 2)).reshape(D, 3 * D)
        else:
            m[k] = v
    return m


# ---------------------------------------------------------------------------
# device program
# ---------------------------------------------------------------------------

def build_program(cfg):
    import concourse.bass as bass
    import concourse.bacc as bacc
    import concourse.mybir as mybir
    import concourse.tile as tile

    f32 = mybir.dt.float32
    bf16 = mybir.dt.bfloat16
    AF = mybir.ActivationFunctionType
    ALU = mybir.AluOpType

    T, NW, OFF, RING = cfg.T, cfg.NW, cfg.OFF, cfg.RING
    C, D, KS, B, PAD = cfg.C, cfg.D, cfg.KS, cfg.B, cfg.PAD
    L, EL, ER, NE = cfg.L, cfg.EL, cfg.ER, cfg.NE
    MAXA = cfg.MAXA
    h_dt = f32 if cfg.h_fp32 else bf16
    g_dt = f32 if cfg.gate_f32 else bf16
    m_dt = f32 if cfg.mt_f32 else bf16
    u_dt = f32 if (cfg.h_fp32 or cfg.gate_f32) else bf16  # h-update intermediates

    # bias column indices
    EVB, C1B, C2B, C3B, SR, SZ, BHN, BIN, BVF, NSZ = range(10)

    nc = bacc.Bacc("TRN2", debug=False, num_devices=cfg.n_cores)

    d_xeT = nc.dram_tensor("xeT", [C, T * B], f32, kind="ExternalInput")
    d_tembT = nc.dram_tensor("tembT", [D, L * B], bf16, kind="ExternalInput")
    d_wval = nc.dram_tensor("wval", [C, D], f32, kind="ExternalInput")
    d_convW = nc.dram_tensor("convW", [D, 3 * KS * D], bf16, kind="ExternalInput")
    d_wiT = nc.dram_tensor("wiT", [D, 3 * D], bf16, kind="ExternalInput")
    d_whT = nc.dram_tensor("whT", [D, 3 * D], bf16, kind="ExternalInput")
    d_wvfT = nc.dram_tensor("wvfT", [D, D], bf16, kind="ExternalInput")
    d_fcT = nc.dram_tensor("fcT", [D, C], h_dt, kind="ExternalInput")
    d_biases = nc.dram_tensor("biases", [D, 10], f32, kind="ExternalInput")
    d_fcb = nc.dram_tensor("fcb", [C, 1], f32, kind="ExternalInput")
    d_out = nc.dram_tensor("outT", [C, NW * B], f32, kind="ExternalOutput")

    def cap(tile_ap, off, dims):
        """Custom AP relative to a pool tile: dims = [(step, count), ...]."""
        part = list(tile_ap.ap)[0]
        return bass.AP(tile_ap.tensor, tile_ap.offset + off, [part] + dims)

    with tile.TileContext(nc) as tc:
        with (
            tc.tile_pool(name="persist", bufs=1) as pp,
            tc.tile_pool(name="work", bufs=2) as wp,
            tc.tile_pool(name="ps2", bufs=2, space="PSUM") as ps2,
            tc.tile_pool(name="ps1", bufs=1, space="PSUM") as ps1,
        ):
            # ---------------- persistent tiles ----------------
            eg = pp.tile([D, L * B], bf16, tag="eg")
            c1g = pp.tile([D, L * B], bf16, tag="c1g")
            c2g = pp.tile([D, L * B], bf16, tag="c2g")
            c3g = pp.tile([D, L * B], bf16, tag="c3g")
            ering = pp.tile([D, RING * NE * B], bf16, tag="ering")
            H = pp.tile([D, NW * B], h_dt, tag="H")
            Hb = pp.tile([D, NW * B], bf16, tag="Hb", name="Hb") if cfg.h_fp32 else H
            ttail = pp.tile([D, NW * B], bf16, tag="ttail")
            xe = pp.tile([C, T * B], f32, tag="xe")
            wval = pp.tile([C, D], f32, tag="wval")
            cw = pp.tile([D, 3 * KS * D], bf16, tag="cw")
            wiT = pp.tile([D, 3 * D], bf16, tag="wiT")
            whT = pp.tile([D, 3 * D], bf16, tag="whT")
            wvfT = pp.tile([D, D], bf16, tag="wvfT")
            fcT = pp.tile([D, C], h_dt, tag="fcT")
            bias = pp.tile([D, 10], f32, tag="bias")
            fcb = pp.tile([C, 1], f32, tag="fcb")
            outsb = pp.tile([C, NW * B], f32, tag="outsb")

            nc.sync.dma_start(xe[:], d_xeT[:])
            nc.sync.dma_start(wval[:], d_wval[:])
            nc.sync.dma_start(cw[:], d_convW[:])
            nc.sync.dma_start(wiT[:], d_wiT[:])
            nc.sync.dma_start(whT[:], d_whT[:])
            nc.sync.dma_start(wvfT[:], d_wvfT[:])
            nc.sync.dma_start(fcT[:], d_fcT[:])
            nc.sync.dma_start(bias[:], d_biases[:])
            nc.sync.dma_start(fcb[:], d_fcb[:])
            nc.sync.dma_start(ttail[:], d_tembT[:, T * B:])

            nc.gpsimd.memset(H[:], 0.0)
            if cfg.h_fp32:
                nc.gpsimd.memset(Hb[:], 0.0)

            def bias_ap(i):
                return bias[:, i:i + 1]

            def conv_lhsT(layer, k):
                i = layer * KS + k
                return cw[:, i * D:(i + 1) * D]

            PSW = max(((MAXA + 1) // 2) * B, 512)
            _ps_cycle = ["r", "z", "ghn"]
            _ps_i = [0]

            def next_ps(width):
                tag = _ps_cycle[_ps_i[0] % len(_ps_cycle)]
                _ps_i[0] += 1
                return ps2.tile([D, PSW], f32, tag=tag, name="ps_" + tag)

            def conv_group(ps, wcols, layer, t0, cnt, vlo, vhi, src_of):
                """Accumulate conv taps for output positions [t0, t0+cnt) into
                ps[:, :cnt*B]. src_of(t, n) -> rhs AP for n consecutive input
                positions starting at t. Valid input positions: [vlo, vhi]."""
                plan = []
                for k in [PAD] + [k for k in range(KS) if k != PAD]:
                    d = k - PAD
                    lo = max(t0, vlo - d)
                    hi = min(t0 + cnt, vhi - d + 1)
                    if hi > lo:
                        plan.append((k, lo, hi))
                assert plan[0][1] == t0 and plan[0][2] == t0 + cnt
                for i, (k, lo, hi) in enumerate(plan):
                    nc.tensor.matmul(
                        ps[:, (lo - t0) * B:(hi - t0) * B],
                        conv_lhsT(layer, k),
                        src_of(lo + k - PAD, hi - lo),
                        start=(i == 0), stop=(i == len(plan) - 1))

            def eg_of(p, n):
                return eg[:, p * B:(p + n) * B]

            def c1g_of(p, n):
                return c1g[:, p * B:(p + n) * B]

            def c2g_of(p, n):
                return c2g[:, p * B:(p + n) * B]

            # ---------------- init: value embedding for positions [0, T) ----
            GP = 512 // B  # positions per psum group
            for p0 in range(0, T, GP):
                cnt = min(GP, T - p0)
                pe = next_ps(cnt * B)
                nc.tensor.matmul(pe[:, :cnt * B], wval[:],
                                 xe[:, p0 * B:(p0 + cnt) * B],
                                 start=True, stop=True)
                tb = wp.tile([D, GP * B], bf16, tag="tstream")
                nc.sync.dma_start(tb[:, :cnt * B],
                                  d_tembT[:, p0 * B:(p0 + cnt) * B])
                nc.vector.scalar_tensor_tensor(
                    eg[:, p0 * B:(p0 + cnt) * B], pe[:, :cnt * B],
                    bias_ap(EVB), tb[:, :cnt * B], ALU.add, ALU.add)

            # ---------------- init: global convs over [0, T) ---------------
            def glob_conv(layer, dst, src_of, plo, phi, vlo, vhi, bcol):
                for p0 in range(plo, phi + 1, GP):
                    cnt = min(GP, phi + 1 - p0)
                    ps = next_ps(cnt * B)
                    conv_group(ps, cnt * B, layer, p0, cnt, vlo, vhi, src_of)
                    nc.scalar.activation(dst[:, p0 * B:(p0 + cnt) * B],
                                         ps[:, :cnt * B], AF.Relu,
                                         bias=bias_ap(bcol))

            glob_conv(0, c1g, eg_of, PAD, T - 1 - PAD, 0, T - 1, C1B)
            glob_conv(1, c2g, c1g_of, 2 * PAD, T - 1 - 2 * PAD,
                      PAD, T - 1 - PAD, C2B)
            glob_conv(2, c3g, c2g_of, 3 * PAD, T - 1 - 3 * PAD,
                      2 * PAD, T - 1 - 2 * PAD, C3B)

            # ---------------- edge computation ----------------------------
            S1L = EL + 2 * PAD   # conv1 span needed for a left edge
            S2L = EL + PAD

            def edge_left_stages(w):
                """Window w conv3 outputs t in [0, EL) -> ring slot, as three
                separately emittable stages."""
                box = {}

                def st1():
                    s1 = wp.tile([D, S1L * B], bf16, tag="s1", name="s1")
                    box["s1"] = s1
                    for t0 in range(0, S1L, GP):
                        cnt = min(GP, S1L - t0)
                        ps = next_ps(cnt * B)
                        conv_group(ps, cnt * B, 0, t0, cnt, 0, T - 1,
                                   lambda t, n: eg[:, (w + t) * B:(w + t + n) * B])
                        nc.scalar.activation(s1[:, t0 * B:(t0 + cnt) * B],
                                             ps[:, :cnt * B], AF.Relu,
                                             bias=bias_ap(C1B))

                def st2():
                    s1 = box["s1"]
                    s2 = wp.tile([D, S2L * B], bf16, tag="s2", name="s2")
                    box["s2"] = s2
                    for t0 in range(0, S2L, GP):
                        cnt = min(GP, S2L - t0)
                        ps = next_ps(cnt * B)
                        conv_group(ps, cnt * B, 1, t0, cnt, 0, S1L - 1,
                                   lambda t, n: s1[:, t * B:(t + n) * B])
                        nc.scalar.activation(s2[:, t0 * B:(t0 + cnt) * B],
                                             ps[:, :cnt * B], AF.Relu,
                                             bias=bias_ap(C2B))

                def st3():
                    s2 = box["s2"]
                    ps = next_ps(EL * B)
                    conv_group(ps, EL * B, 2, 0, EL, 0, S2L - 1,
                               lambda t, n: s2[:, t * B:(t + n) * B])
                    base = ((w % RING) * NE + 0) * B
                    nc.scalar.activation(
                        cap(ering, base, [(1, EL * B)]),
                        ps[:, :EL * B], AF.Relu, bias=bias_ap(C3B))

                return st1, st2, st3

            def edge_left(w):
                for st in edge_left_stages(w):
                    st()

            def edge_right(w):
                """Window w conv3 outputs t in [T-ER, T) -> ring slot."""
                t1lo = T - ER - 2 * PAD
                s1 = wp.tile([D, S1L * B], bf16, tag="s1r")
                for i0 in range(0, S1L, GP):
                    cnt = min(GP, S1L - i0)
                    ps = next_ps(cnt * B)
                    conv_group(ps, cnt * B, 0, t1lo + i0, cnt, 0, T - 1,
                               lambda t, n: eg[:, (w + t) * B:(w + t + n) * B])
                    nc.scalar.activation(s1[:, i0 * B:(i0 + cnt) * B],
                                         ps[:, :cnt * B], AF.Relu,
                                         bias=bias_ap(C1B))
                t2lo = T - ER - PAD
                s2 = wp.tile([D, S2L * B], bf16, tag="s2r")
                for i0 in range(0, S2L, GP):
                    cnt = min(GP, S2L - i0)
                    ps = next_ps(cnt * B)
                    conv_group(ps, cnt * B, 1, t2lo + i0, cnt,
                               t1lo, T - 1,
                               lambda t, n: s1[:, (t - t1lo) * B:(t - t1lo + n) * B])
                    nc.scalar.activation(s2[:, i0 * B:(i0 + cnt) * B],
                                         ps[:, :cnt * B], AF.Relu,
                                         bias=bias_ap(C2B))
                ps = next_ps(ER * B)
                conv_group(ps, ER * B, 2, T - ER, ER, t2lo, T - 1,
                           lambda t, n: s2[:, (t - t2lo) * B:(t - t2lo + n) * B])
                base = ((w % RING) * NE + EL) * B
                nc.scalar.activation(
                    cap(ering, base, [(1, ER * B)]),
                    ps[:, :ER * B], AF.Relu, bias=bias_ap(C3B))

            for w in range(min(RING, NW)):
                edge_left(w)
            edge_right(0)

            # ---------------- pipelined GRU ticks --------------------------
            def gx_segments(act, tau):
                """[(col0, ncols, rhs_builder)] covering the active chains."""
                segs = []
                i = 0
                while i < len(act):
                    w, t = act[i]
                    if EL <= t <= T - 1 - ER:
                        j = i
                        while (j + 1 < len(act)
                               and EL <= act[j + 1][1] <= T - 1 - ER):
                            j += 1
                        n = j - i + 1
                        base = (tau - (OFF - 1) * w) * B
                        if n == 1:
                            segs.append((i, n, c3g[:, base:base + B]))
                        else:
                            segs.append((i, n, cap(
                                c3g, base, [((OFF - 1) * B, n), (1, B)])))
                        i = j + 1
                    else:
                        base = ((w % RING) * NE + cfg.emap(t)) * B
                        segs.append((i, 1, cap(ering, base, [(1, B)])))
                        i += 1
                return segs

            fixup_at = {}
            if NW > 1:
                for v in range(NW - 1):
                    fixup_at[OFF * v + T - 1] = v

            def emit_group(act, tau):
                """One tick step for a (sub)set of active chains; chains in
                `act` are w-descending with constant w-step `ws`."""
                nA = len(act)
                W = nA * B
                ws = act[0][0] - act[1][0] if nA > 1 else 1
                whi = act[0][0]
                slo = NW - 1 - whi

                def h_ap(t):
                    if nA == 1:
                        return t[:, slo * B:(slo + 1) * B]
                    return cap(t, slo * B, [(ws * B, nA), (1, B)])

                # gx rhs segments
                segs = []
                i = 0
                while i < nA:
                    w, t = act[i]
                    if EL <= t <= T - 1 - ER:
                        j = i
                        while (j + 1 < nA
                               and EL <= act[j + 1][1] <= T - 1 - ER):
                            j += 1
                        n = j - i + 1
                        base = (tau - (OFF - 1) * w) * B
                        if n == 1:
                            segs.append((i, n, c3g[:, base:base + B]))
                        else:
                            segs.append((i, n, cap(
                                c3g, base, [(ws * (OFF - 1) * B, n), (1, B)])))
                        i = j + 1
                    else:
                        base = ((w % RING) * NE + cfg.emap(t)) * B
                        segs.append((i, 1, cap(ering, base, [(1, B)])))
                        i += 1

                pr = ps2.tile([D, PSW], f32, tag="r", name="pr")
                pz = ps2.tile([D, PSW], f32, tag="z", name="pz")
                pn = ps2.tile([D, PSW], f32, tag="ghn", name="pn")
                px = ps1.tile([D, PSW], f32, tag="gxn", name="px")

                # gx matmuls (h-independent)
                for g, ps in ((0, pr), (1, pz), (2, px)):
                    for si, (i0, n, rhs) in enumerate(segs):
                        nc.tensor.matmul(
                            ps[:, i0 * B:(i0 + n) * B],
                            wiT[:, g * D:(g + 1) * D], rhs,
                            start=(si == 0),
                            stop=(g == 2 and si == len(segs) - 1))
                hb_sl = h_ap(Hb)
                # recurrent matmuls: r first (heads the serial chain), n next
                nc.tensor.matmul(pr[:, :W], whT[:, 0:D], hb_sl,
                                 start=False, stop=True)
                nc.tensor.matmul(pn[:, :W], whT[:, 2 * D:3 * D], hb_sl,
                                 start=True, stop=True)
                nc.tensor.matmul(pz[:, :W], whT[:, D:2 * D], hb_sl,
                                 start=False, stop=True)

                rz = wp.tile([D, 2 * MAXA * B], g_dt, tag="rz")
                r_sl = rz[:, 0:W]
                z_sl = rz[:, MAXA * B:MAXA * B + W]
                h_sl = h_ap(H)
                # critical chain: sigmoid(r) -> m -> tt -> tanh -> q -> h'
                nc.scalar.activation(r_sl, pr[:, :W], AF.Sigmoid,
                                     bias=bias_ap(SR))
                m = wp.tile([D, MAXA * B], m_dt, tag="m")
                nc.vector.scalar_tensor_tensor(m[:, :W], pn[:, :W],
                                               bias_ap(BHN), r_sl,
                                               ALU.add, ALU.mult)
                tt = wp.tile([D, MAXA * B], m_dt, tag="tt")
                nc.vector.tensor_add(tt[:, :W], m[:, :W], px[:, :W])
                # off-chain: z, z' = 1-z, zh = z*h
                nc.scalar.activation(z_sl, pz[:, :W], AF.Sigmoid,
                                     bias=bias_ap(SZ))
                zp = wp.tile([D, MAXA * B], g_dt, tag="zp")
                nc.vector.tensor_scalar(
                    out=zp[:, :W], in0=z_sl, scalar1=-1.0, scalar2=1.0,
                    op0=ALU.mult, op1=ALU.add)
                zh = wp.tile([D, MAXA * B], u_dt, tag="zh")
                nc.vector.tensor_mul(zh[:, :W], z_sl, h_sl)
                n_t = wp.tile([D, MAXA * B], g_dt, tag="n")
                nc.scalar.activation(n_t[:, :W], tt[:, :W], AF.Tanh,
                                     bias=bias_ap(BIN))
                q_t = wp.tile([D, MAXA * B], u_dt, tag="q")
                nc.vector.tensor_mul(q_t[:, :W], zp[:, :W], n_t[:, :W])
                nc.vector.tensor_add(h_sl, q_t[:, :W], zh[:, :W])
                if cfg.h_fp32:
                    nc.vector.tensor_copy(hb_sl, h_sl)

            pending = {}

            def sched(t, fn):
                pending.setdefault(min(t, cfg.TICKS - 1), []).append(fn)

            for tau in range(cfg.TICKS):
                whi = min(tau // OFF, NW - 1)
                wlo = max((tau - (T - 1) + OFF - 1) // OFF, 0)
                act = [(w, tau - OFF * w) for w in range(whi, wlo - 1, -1)]
                for grp in (0, 1):
                    act_g = [p for p in act if p[0] % 2 == grp]
                    if act_g:
                        emit_group(act_g, tau)
                for fn in pending.pop(tau, []):
                    fn()

                # fixup after chain v finishes
                v = fixup_at.get(tau)
                if v is not None:
                    sv = NW - 1 - v
                    pe = ps1.tile([D, 512], f32, tag="conv")
                    nc.tensor.matmul(pe[:, :B], wvfT[:],
                                     Hb[:, sv * B:(sv + 1) * B],
                                     start=True, stop=True)
                    nc.vector.scalar_tensor_tensor(
                        eg[:, (T + v) * B:(T + v + 1) * B], pe[:, :B],
                        bias_ap(BVF), ttail[:, v * B:(v + 1) * B],
                        ALU.add, ALU.add)
                    # global conv extensions (one new position per layer)
                    for layer, dst, src_of, bcol in (
                            (0, c1g, eg_of, C1B), (1, c2g, c1g_of, C2B),
                            (2, c3g, c2g_of, C3B)):
                        p1 = T + v - (layer + 1) * PAD
                        ps = ps1.tile([D, 512], f32, tag="conv")
                        conv_group(ps, B, layer, p1, 1, 0, L, src_of)
                        nc.scalar.activation(dst[:, p1 * B:(p1 + 1) * B],
                                             ps[:, :B], AF.Relu,
                                             bias=bias_ap(bcol))
                    sched(tau + 1, lambda v=v: edge_right(v + 1))
                    if v + RING < NW:
                        sts = edge_left_stages(v + RING)
                        for di, st in enumerate(sts):
                            sched(tau + 2 + di, st)

            for tq in sorted(pending):
                for fn in pending.pop(tq, []):
                    fn()

            # ---------------- final fc over all stashed h ------------------
            for c0 in range(0, NW * B, 512):
                cnt = min(512, NW * B - c0)
                pf = ps1.tile([C, 512], f32, tag="conv")
                nc.tensor.matmul(pf[:, :cnt], fcT[:], H[:, c0:c0 + cnt],
                                 start=True, stop=True)
                nc.scalar.activation(outsb[:, c0:c0 + cnt], pf[:, :cnt],
                                     AF.Identity, bias=fcb[:])
            nc.sync.dma_start(d_out[:], outsb[:])

    nc.compile()
    return nc


# ---------------------------------------------------------------------------
# top-level entry
# ---------------------------------------------------------------------------

_CACHE = {}


def _get_program(cfg):
    key = (cfg.T, cfg.NW, cfg.OFF, cfg.RING, cfg.h_fp32, cfg.gate_f32,
           cfg.mt_f32, cfg.n_cores)
    if key not in _CACHE:
        _CACHE[key] = build_program(cfg)
    return _CACHE[key]


def unshard(cfg, outs):
    """outs: list of per-core outT [C, NW*B] -> full [Bfull, NW, C]."""
    full = np.zeros((cfg.B * cfg.n_cores, cfg.NW, cfg.C), np.float32)
    for core, o in enumerate(outs):
        ot = np.asarray(o).reshape(cfg.C, cfg.NW, cfg.B)
        # slot s corresponds to window v = NW-1-s
        full[core * cfg.B:(core + 1) * cfg.B] = ot[:, ::-1, :].transpose(2, 1, 0)
    return full


def kernel(**inputs):
    from concourse.bass_utils import run_bass_kernel_spmd

    cfg = REAL
    nc = _get_program(cfg)
    shared = host_shared(cfg, inputs)
    temb = host_temb(cfg, inputs)
    in_maps = [host_core_inputs(cfg, inputs, shared, temb, c)
               for c in range(cfg.n_cores)]
    res = run_bass_kernel_spmd(nc, in_maps, list(range(cfg.n_cores)))
    outs = [res.results[c]["outT"] for c in range(cfg.n_cores)]
    return unshard(cfg, outs)
